# revision 58
# baseline (speedup 1.0000x reference)
"""Trainium2 Bass kernel for nn_AutoEncoder (scatter_memory).

Strategy (z-slab shard over 8 cores, all 16 batch rows per core):
- Host: linearize+sort indices, segment-combine duplicate voxels (bf16),
  slab-select 17 z-planes per core (1 overlap plane; core 7 duplicates its
  top plane so its overlap z-diff is exactly zero), pack occupied 8-voxel
  groups into 256B tokens [16 batch x 8 x] and emit per-window token/idx
  arrays for dma_scatter_add (int16 idx, -1 tail padding -> no descriptors).
- Device per core: 5 dma_scatter_add calls (~34K 256B descriptors total,
  4 SWDGE queues) build the slab grid [34816 groups, 128] bf16 in HBM;
  chunked strided readback puts it in SBUF as [y=128, z*2048]; DVE computes
  bf16 diffs along x/y/z (y via partition-shifted SBUF-SBUF DMA copy) and
  packed-innermost abs/square reduction passes (ACT does the squares);
  per-batch partials [128, 32] go back to the host for the final sum+scale.
"""
import sys
import numpy as np
from ml_dtypes import bfloat16

sys.path.insert(0, '/opt/trn_rl_repo')

N_CORES = 8
B, N, XS = 16, 1_000_000, 128
V = XS * XS * XS
PLANE = XS * XS          # 16384 voxels per z-plane
GPP = PLANE // 8         # 2048 groups (256B rows) per plane
NZ = 17                  # planes per slab (16 + 1 overlap)
NG = NZ * GPP            # 34816 grid rows per core
WIN = 7936               # groups per scatter window
CAP = 8064               # tokens per scatter call (63*128)
NCALL = 5
NUM_PAIRS = float(2 * XS * XS - 2 * XS)

_compiled = {}
_MODE = __import__('os').environ.get('KMODE', 'full')  # full|scat|comp|sim debug
_VAR = __import__('os').environ.get('KVAR', 'base')
# base|noy|dvesq|nosq|y128|nox|nocross|noz


def _host_prep(values, indices):
    idx = indices.astype(np.int64)
    lin = (idx[:, 0] * XS + idx[:, 1]) * XS + idx[:, 2]
    order = np.argsort(lin, kind='stable')
    lin_s = lin[order]
    uniq, starts = np.unique(lin_s, return_index=True)
    sums = np.add.reduceat(values[:, order], starts, axis=1).astype(bfloat16)

    Fs, IXs, CNs = [], [], []
    for c in range(N_CORES):
        z0 = 16 * c
        lo = np.searchsorted(uniq, z0 * PLANE)
        hi = np.searchsorted(uniq, min(z0 + NZ, XS) * PLANE)
        vloc = uniq[lo:hi] - z0 * PLANE
        sv = sums[:, lo:hi]
        if c == N_CORES - 1:
            m = vloc >= 15 * PLANE
            vloc = np.concatenate([vloc, vloc[m] + PLANE])
            sv = np.concatenate([sv, sv[:, m]], axis=1)
        g = vloc // 8
        xl = vloc % 8
        gu = np.unique(g)
        tok = np.searchsorted(gu, g)
        P = np.zeros((len(gu), 16, 8), bfloat16)
        P[tok, :, xl] = sv.T
        P = P.reshape(len(gu), 128)

        F = np.zeros((NCALL, CAP, 128), bfloat16)
        IX = np.full((NCALL, CAP), -1, np.int16)
        for w in range(NCALL):
            glo, ghi = w * WIN, min((w + 1) * WIN, NG)
            a, b = np.searchsorted(gu, [glo, ghi])
            n = b - a
            assert n <= CAP, (c, w, n)
            F[w, :n] = P[a:b]
            IX[w, :n] = (gu[a:b] - glo).astype(np.int16)
        Fw = F.reshape(NCALL, 63, 128, 128).transpose(2, 0, 1, 3).reshape(128, -1)
        IXw = IX.reshape(NCALL, 504, 16).transpose(2, 0, 1).reshape(16, -1)
        Fs.append(np.ascontiguousarray(Fw))
        IXs.append(np.ascontiguousarray(np.tile(IXw, (8, 1))))
        CNs.append((IX >= 0).sum(axis=1).astype(np.int32).reshape(1, NCALL))
    return Fs, IXs, CNs


def _build():
    from concourse import bass, bacc, mybir
    from concourse import library_config
    from contextlib import ExitStack

    NQ = 4
    nc = bacc.Bacc("TRN2", target_bir_lowering=False, debug=False,
                   num_devices=N_CORES, num_swdge_queues=NQ)
    BF = mybir.dt.bfloat16
    F32 = mybir.dt.float32
    A = mybir.AluOpType
    AX = mybir.AxisListType
    f_d = nc.dram_tensor("fsrc", [128, NCALL * 63 * 128], BF, kind="ExternalInput")
    idx_d = nc.dram_tensor("idx16", [128, NCALL * 504], mybir.dt.int16,
                           kind="ExternalInput")
    cnt_in = nc.dram_tensor("cnts", [1, NCALL], mybir.dt.int32,
                            kind="ExternalInput")
    grid_d = nc.dram_tensor("grid", [NG, 128], BF, kind="ExternalOutput")
    out_d = nc.dram_tensor("out", [128, 32], F32, kind="ExternalOutput")
    cnt_d = nc.dram_tensor("cnt", [1, 2], mybir.dt.uint32, kind="ExternalOutput")

    NCH = 4                       # 4-plane compute chunks
    KINDS = {'noy': (0, 1, 3), 'nox': (1, 2, 3), 'nocross': (0, 2, 3),
             'noz': (0, 1, 2)}.get(_VAR, (0, 1, 2, 3))
    Y128 = _VAR == 'y128'  # y ops on 128 partitions (dup row -> zero diff)
    SLOTS = [(c, k) for c in range(NCH) for k in KINDS]
    NSLOT = len(SLOTS)            # chunks x (x, cross[, y], z)
    YSLOT = {c: s for s, (c_, k) in enumerate(SLOTS) if k == 2
             for c in [c_]}
    SZ = {0: 7168, 1: 960, 2: 8192, 3: 8192}   # D elems per slot kind

    with ExitStack() as st:
        block = st.enter_context(nc.Block())
        io = st.enter_context(nc.semaphore("io"))
        start_sem = st.enter_context(nc.semaphore("startc"))
        scat_done = st.enter_context(nc.semaphore("scatdone"))
        gl = [nc.alloc_semaphore(f"gl{i}") for i in range(5)]
        yc = [nc.alloc_semaphore(f"yc{i}") for i in range(4)]
        dsub = st.enter_context(nc.semaphore("dsub"))
        asq = st.enter_context(nc.semaphore("asq"))
        psd = st.enter_context(nc.semaphore("psd"))
        fin = st.enter_context(nc.semaphore("fin"))
        flgi = st.enter_context(nc.semaphore("flgi"))
        ared = st.enter_context(nc.semaphore("ared"))
        outd = st.enter_context(nc.semaphore("outd"))
        qs = [nc.alloc_semaphore(f"q{w}") for w in range(NCALL)]

        FLG = st.enter_context(nc.sbuf_tensor("FLG", [1, 2], mybir.dt.uint32))
        # --- phase-1 tensors; freed before G is allocated (scatter DMA
        # completion is ordered before the readback via scat_done) ---
        with (nc.sbuf_tensor("F", [128, NCALL * 63 * 128], BF) as FT,
              nc.sbuf_tensor("IX", [128, NCALL * 504], mybir.dt.int16) as IX,
              nc.sbuf_tensor("CN", [1, NCALL], mybir.dt.int32) as CN):

            @block.gpsimd
            def _(gp):
                gp.load_library(library_config.mlp)
                gp.dma_start(FT[:], f_d[:]).then_inc(io, 16)
                gp.dma_start(IX[:], idx_d[:]).then_inc(io, 16)
                gp.dma_start(CN[:], cnt_in[:]).then_inc(io, 16)
                gp.wait_ge(io, 48)
                gp.wait_ge(flgi, 1)
                gp.sem_inc(start_sem, 1)
                nreg = gp.alloc_register("nidx")
                for w in range(NCALL if _MODE != 'comp' else 0):
                    r0, r1 = w * WIN, min((w + 1) * WIN, NG)
                    in_ap = FT[:, w * 63 * 128:(w + 1) * 63 * 128].rearrange(
                        "p (t e) -> p t e", e=128)
                    gp.reg_load(nreg, CN[0:1, w:w + 1])
                    gp.dma_scatter_add(
                        grid_d[r0:r1, :], in_ap,
                        IX[:, w * 504:(w + 1) * 504],
                        CAP, nreg, 128, elem_step=128, queue_num=w % NQ,
                    ).then_inc(qs[w], 16)
                for w in range(NCALL if _MODE != 'comp' else 0):
                    gp.wait_ge(qs[w], 16)
                gp.sem_inc(scat_done, 1)

        G = st.enter_context(nc.sbuf_tensor("G", [128, NG], BF))
        GY = [st.enter_context(nc.sbuf_tensor(f"GY{i}", [128, 8192], BF))
              for i in range(NCH)]
        D = [st.enter_context(nc.sbuf_tensor(f"D{i}", [128, 8192], BF))
             for i in range(2)]
        E = [st.enter_context(nc.sbuf_tensor(f"E{i}", [128, 8192], BF))
             for i in range(2)]
        PA = st.enter_context(nc.sbuf_tensor("PA", [128, NSLOT * 16], BF))
        PS = st.enter_context(nc.sbuf_tensor("PS", [128, NSLOT * 16], BF))
        OT = st.enter_context(nc.sbuf_tensor("OT", [128, 32], F32))

        @block.sync
        def _(sy):
            sy.wait_ge(scat_done, 1)
            if _MODE == 'scat':
                sy.dma_start(out_d[:], OT[:]).then_inc(outd, 16)
                sy.wait_ge(outd, 16)
                return
            for c in range(NCH + 1):
                r0 = c * 4 * GPP
                nz = 4 if c < NCH else 1
                v = grid_d[r0:r0 + nz * GPP, :].rearrange(
                    "(z y q) u -> y z (q u)", z=nz, y=128, q=16)
                dst = G[:, c * 8192:c * 8192 + nz * 2048].rearrange(
                    "p (z qu) -> p z qu", z=nz, qu=2048)
                sy.dma_start(dst, v).then_inc(gl[c], 16)
                if c < NCH and 2 in KINDS:
                    for j in range(4):
                        zz = 4 * c + j
                        v = grid_d[zz * GPP + 16:(zz + 1) * GPP, :].rearrange(
                            "(y q) u -> y (q u)", y=127, q=16)
                        sy.dma_start(GY[c][0:127, j * 2048:(j + 1) * 2048], v
                                     ).then_inc(yc[c], 16)
            sy.wait_ge(fin, 2)
            sy.dma_start(out_d[:], OT[:]).then_inc(outd, 16)
            sy.wait_ge(outd, 16)

        @block.vector
        def _(ve):
            ve.memset(FLG[:], 0).then_inc(flgi, 1)
            ve.wait_ge(scat_done, 1)
            ve.wait_ge(flgi, 1)
            ve.memset(FLG[0:1, 0:1], 1)
            if _MODE == 'scat':
                ve.memset(FLG[0:1, 1:2], 1)
                return
            with nc.allow_low_precision(reason="bf16 slot partials, f32 final"):
                ve.memset(PA[:], 0).then_inc(ared, 1)
                ve.memset(PS[:], 0).then_inc(ared, 1)
                ve.wait_ge(ared, 2)

                def gview(c):
                    return G[:, c * 8192:(c + 1) * 8192].rearrange(
                        "p (z xg b xl) -> p z xg b xl", z=4, xg=16, b=16, xl=8)

                def emit_sub(s):
                    c, k = SLOTS[s]
                    d = D[s % 2]
                    if k in (0, 1):
                        ve.wait_ge(gl[c], 16)
                    elif k == 2:
                        ve.wait_ge(yc[c], 64)
                    else:
                        ve.wait_ge(gl[c], 16)
                        ve.wait_ge(gl[c + 1], 16)
                    if s >= 2:
                        # D[s%2] reuse: slot s-2's square (ACT read) and
                        # abs-reduce (DVE read) must have completed
                        if _VAR != 'nosq':
                            ve.wait_ge(asq, s - 1)
                        ve.wait_ge(ared, s + 1)
                    if k == 0:
                        gv = gview(c)
                        dv = d[:, 0:7168].rearrange(
                            "p (z xg b xl) -> p z xg b xl",
                            z=4, xg=16, b=16, xl=7)
                        ins = ve.tensor_tensor(dv, gv[:, :, :, :, 1:8],
                                               gv[:, :, :, :, 0:7],
                                               op=A.subtract)
                    elif k == 1:
                        gv = gview(c)
                        dv = d[:, 0:960].rearrange(
                            "p (z xg b one) -> p z xg b one",
                            z=4, xg=15, b=16, one=1)
                        ins = ve.tensor_tensor(dv, gv[:, :, 1:16, :, 0:1],
                                               gv[:, :, 0:15, :, 7:8],
                                               op=A.subtract)
                    elif k == 2:
                        ins = ve.tensor_tensor(
                            d[0:127, :], GY[c][0:127, :],
                            G[0:127, c * 8192:(c + 1) * 8192], op=A.subtract)
                    else:
                        b0 = c * 8192 + 2048
                        ins = ve.tensor_tensor(
                            d[:], G[:, b0:b0 + 8192],
                            G[:, c * 8192:c * 8192 + 8192], op=A.subtract)
                    ins.then_inc(dsub, 1)

                def emit_red(s, src, dst, abs_):
                    k = SLOTS[s][1]
                    if k == 0:
                        v = src[:, 0:7168].rearrange(
                            "p (z xg b xl) -> p b z xg xl",
                            z=4, xg=16, b=16, xl=7)
                        ax = AX.XYZ
                        pp = 128
                    elif k == 1:
                        v = src[:, 0:960].rearrange(
                            "p (z xg b) -> p b z xg", z=4, xg=15, b=16)
                        ax = AX.XY
                        pp = 128
                    elif k == 2:
                        pp = 128 if Y128 else 127
                        v = src[0:pp, :].rearrange(
                            "p (zxg b xl) -> p b zxg xl", zxg=64, b=16, xl=8)
                        ax = AX.XY
                    else:
                        v = src[:, :].rearrange(
                            "p (zxg b xl) -> p b zxg xl", zxg=64, b=16, xl=8)
                        ax = AX.XY
                        pp = 128
                    return ve.tensor_reduce(dst[0:pp, s * 16:(s + 1) * 16], v,
                                            axis=ax, op=A.add,
                                            apply_absolute_value=abs_)

                for s in range(NSLOT):
                    emit_sub(s)
                    ve.wait_ge(dsub, s + 1)
                    emit_red(s, D[s % 2], PA, True).then_inc(ared, 1)
                    if _VAR == 'dvesq':
                        c, k = SLOTS[s]
                        sz = SZ[k]
                        pp = 127 if (k == 2 and not Y128) else 128
                        if s >= 2:
                            ve.wait_ge(psd, s - 1)  # E reuse: sqred(s-2) done
                        ve.tensor_tensor(E[s % 2][0:pp, 0:sz],
                                         D[s % 2][0:pp, 0:sz],
                                         D[s % 2][0:pp, 0:sz],
                                         op=A.mult).then_inc(asq, 1)
                    if _VAR != 'nosq' and s >= 1:
                        ve.wait_ge(asq, s)
                        emit_red(s - 1, E[(s - 1) % 2], PS, False
                                 ).then_inc(psd, 1)
                if _VAR != 'nosq':
                    ve.wait_ge(asq, NSLOT)
                    emit_red(NSLOT - 1, E[(NSLOT - 1) % 2], PS, False
                             ).then_inc(psd, 1)

                ve.wait_ge(ared, 2 + NSLOT)
                if _VAR != 'nosq':
                    ve.wait_ge(psd, NSLOT)
                pav = PA[:].rearrange("p (s b) -> p b s", b=16)
                psv = PS[:].rearrange("p (s b) -> p b s", b=16)
                ve.tensor_reduce(OT[:, 0:16], pav, axis=AX.X, op=A.add
                                 ).then_inc(fin, 1)
                ve.tensor_reduce(OT[:, 16:32], psv, axis=AX.X, op=A.add
                                 ).then_inc(fin, 1)
                ve.memset(FLG[0:1, 1:2], 1)

        @block.scalar
        def _(sc):
            if _MODE == 'scat' or _VAR in ('dvesq', 'nosq'):
                return
            for s in range(NSLOT):
                sc.wait_ge(dsub, s + 1)
                if s >= 2:
                    sc.wait_ge(psd, s - 1)
                sz = SZ[SLOTS[s][1]]
                pp = 127 if (SLOTS[s][1] == 2 and not Y128) else 128
                sc.activation(E[s % 2][0:pp, 0:sz], D[s % 2][0:pp, 0:sz],
                              mybir.ActivationFunctionType.Square
                              ).then_inc(asq, 1)

        if _MODE == 'sim':
            nc.compile()
            return nc

        @block.tensor
        def _(te):
            cntr = te.alloc_register("cntr")
            flag = te.alloc_register("flagr")
            nd = te.alloc_register("nd")
            te.reg_mov(cntr, 0)
            te.reg_mov(nd, 1)
            te.wait_ge(start_sem, 1)
            with te.While(lambda: nd):
                te.reg_load(flag, FLG[0:1, 0:1])
                te.reg_alu(nd, flag, 0, A.is_equal)
                te.reg_alu(cntr, cntr, 1, A.add)
            te.reg_save(cnt_d[0:1, 0:1], cntr)
            te.reg_mov(nd, 1)
            with te.While(lambda: nd):
                te.reg_load(flag, FLG[0:1, 1:2])
                te.reg_alu(nd, flag, 0, A.is_equal)
                te.reg_alu(cntr, cntr, 1, A.add)
            te.reg_save(cnt_d[0:1, 1:2], cntr)

    nc.compile()
    return nc


def kernel(values, indices, xsize):
    from concourse.bass_utils import run_bass_kernel_spmd
    values = np.asarray(values, np.float32)
    indices = np.asarray(indices)
    Fs, IXs, CNs = _host_prep(values, indices)
    if 'nc' not in _compiled:
        _compiled['nc'] = _build()
    nc = _compiled['nc']
    in_maps = [{"fsrc": Fs[c], "idx16": IXs[c], "cnts": CNs[c]}
               for c in range(N_CORES)]
    res = run_bass_kernel_spmd(nc, in_maps, list(range(N_CORES)))
    tv = np.zeros(B, np.float64)
    mse = np.zeros(B, np.float64)
    clk = []
    for c in range(N_CORES):
        part = res.results[c]["out"]          # [128, 32]
        tots = part.sum(axis=0, dtype=np.float64)
        tv += tots[0:16]
        mse += tots[16:32]
        cc = res.results[c]["cnt"]
        clk.append((int(cc[0, 0]), int(cc[0, 1])))
    kernel.last_clock_iters = clk
    return (tv / V).astype(np.float32), (mse / NUM_PAIRS).astype(np.float32)
